# revision 11
# baseline (speedup 1.0000x reference)
"""Trainium2 Bass kernel for nn_RSPPLoss (segment-mean prototypes + softmax ratio).

Math (faithful to the reference):
    seg       = where(sub_labels==0, main_labels, C)     # non-main rows dropped
    sums[c]   = sum_{i: seg_i=c} feat[i]                 # [C, D]
    proto_n   = sums / max(||sums||, eps)                # count scale cancels
    present   = ||sums||^2 > eps                         # == counts>0 for real data
    logits    = feat @ proto_n.T                         # [N, C]
    denom[i]  = sum_all_c exp(logits[i,c]) - n_absent    # absent col -> logit 0,
                                                         #   exp(0)=1 exactly
    pos[i]    = exp(feat_i . proto_n[label_i])
    p[i]      = present[label_i] ? pos/denom : 1.0
present[label_i] is recovered from the gathered prototype row itself:
||row||^2 is 1 for present classes (unit vector) and exactly 0 for absent.

Distribution: data-parallel over N across 8 cores; per-core partial
segment-sums in [D, C] layout accumulated in PSUM via one-hot matmuls
(float32r = full-rate 4-byte matmul), AllReduce of the 1MB [D, C] table,
then each core computes its logits shard locally.

Per-core engine plan:
  PE : one-hot segment-sum matmuls, feat-tile transposes, logits matmuls
  ACT: big exp over the PSUM logits with free row-sum accumulation (denom),
       PSUM->SBUF copies, ln/exp for rsqrt
  DVE: one-hot masks (int16 iota vs per-partition fp32 segment id),
       pos dot products (tensor_tensor_reduce), small tail math
  GPSIMD: one dma_gather of all 8192 prototype rows by label
"""

import sys

if "/opt/trn_rl_repo" not in sys.path:
    sys.path.insert(0, "/opt/trn_rl_repo")

import numpy as np

import concourse.bass as bass
import concourse.tile as tile
from concourse import bacc, mybir
from concourse.bass_utils import run_bass_kernel_spmd

N = 65536
D = 128
C = 2048
NCORES = 8
NS = N // NCORES          # rows per core = 8192
T = NS // 128             # row tiles per core = 64
CT = C // 128             # class tiles = 16

f32 = mybir.dt.float32
f32r = mybir.dt.float32r
i16 = mybir.dt.int16
i32 = mybir.dt.int32

P = 128
EPS_PRESENT = 1e-12
EPS_NORM2 = 1e-24


def build_kernel():
    nc = bacc.Bacc("TRN2", target_bir_lowering=False, debug=False, num_devices=NCORES)

    feat = nc.declare_dram_parameter("feat", [NS, D], f32r, isOutput=False)
    seg = nc.declare_dram_parameter("seg", [NS, 1], f32, isOutput=False)
    labels = nc.declare_dram_parameter("labels", [NS, 1], i32, isOutput=False)
    iota = nc.declare_dram_parameter("iota", [P, C], i16, isOutput=False)
    ident = nc.declare_dram_parameter("ident", [P, P], f32r, isOutput=False)
    ones_col = nc.declare_dram_parameter("ones_col", [P, 1], f32, isOutput=False)
    ones_row = nc.declare_dram_parameter("ones_row", [1, P], f32, isOutput=False)

    out_p = nc.declare_dram_parameter("out_p", [NS, 1], f32, isOutput=True)

    # internal DRAM
    ar_in = nc.dram_tensor("ar_in", [P, C], f32)
    ar_out = nc.dram_tensor("ar_out", [P, C], f32)
    table = nc.dram_tensor("table", [C, D], f32)

    with tile.TileContext(nc, num_cores=NCORES) as tc:
        with (
            tc.tile_pool(name="const", bufs=1) as cpool,
            tc.tile_pool(name="persist", bufs=1) as pers,
            tc.tile_pool(name="work", bufs=2) as work,
            tc.tile_pool(name="small", bufs=2) as small,
        ):
            import contextlib
            phase12_psum = contextlib.ExitStack()
            ps_acc = phase12_psum.enter_context(
                tc.tile_pool(name="psum_acc", bufs=1, space="PSUM")
            )
            ps_tr = phase12_psum.enter_context(
                tc.tile_pool(name="psum_tr", bufs=2, space="PSUM")
            )
            # ---------- constants ----------
            iota_t = cpool.tile([P, C], i16)
            nc.sync.dma_start(out=iota_t[:], in_=iota[:, :])
            ident_t = cpool.tile([P, P], f32r)
            nc.sync.dma_start(out=ident_t[:], in_=ident[:, :])
            ones_col_t = cpool.tile([P, 1], f32)
            nc.sync.dma_start(out=ones_col_t[:], in_=ones_col[:, :])
            ones_row_t = cpool.tile([1, P], f32)
            nc.sync.dma_start(out=ones_row_t[:1], in_=ones_row[:, :])
            lab_all = cpool.tile([P, T], i32)

            # ---------- persistent SBUF ----------
            featn = pers.tile([P, NS], f32r)     # natural feat, tile t at cols t*128+
            featT = pers.tile([P, NS], f32r)     # transposed feat (d on partitions)
            protoT = pers.tile([P, C], f32r)     # normalized prototypes, [D, C]
            g_all = pers.tile([P, NS], f32)      # gathered proto rows per sample
            seg_all = pers.tile([P, T], f32)     # seg value per row (partition=row%128)
            denom_all = pers.tile([P, T], f32)
            l_all = pers.tile([P, T], f32)
            zsq_all = pers.tile([P, T], f32)
            nat_all = pers.tile([P, C], f32r)    # phase-2 [C, D] natural tiles
            norm2_all = pers.tile([P, CT], f32)
            pres_all = pers.tile([P, CT], f32)

            # feat DMA: [NS, D] DRAM -> [128, NS] SBUF (row r=t*128+u at (u, t*128+d))
            # in 4 chunks for DMA/compute overlap
            feat_r = feat.rearrange("(t u) d -> u t d", u=P)      # [128, 64, 128]
            featn_3d = featn[:].rearrange("p (t d) -> p t d", d=D)
            for ch in range(4):
                sl = slice(ch * (T // 4), (ch + 1) * (T // 4))
                nc.sync.dma_start(out=featn_3d[:, sl, :], in_=feat_r[:, sl, :])
            seg_r = seg.rearrange("(t u) x -> u t x", u=P)          # [128, 64, 1]
            nc.sync.dma_start(
                out=seg_all[:].unsqueeze(-1), in_=seg_r[:, :, :]
            )
            lab_r = labels.rearrange("(t u) x -> u t x", u=P)
            nc.sync.dma_start(out=lab_all[:].unsqueeze(-1), in_=lab_r[:, :, :])

            # ---------- phase 1: per-core segment sums (transposed [D, C]) ----------
            sumsT_ps = ps_acc.tile([P, C], f32)  # 4 banks, accumulated over T tiles
            for t in range(T):
                fn = featn[:, t * P : (t + 1) * P]
                oh = work.tile([P, C], f32r, tag="onehot")
                nc.vector.tensor_scalar(
                    out=oh[:],
                    in0=iota_t[:],
                    scalar1=seg_all[:, t : t + 1],
                    scalar2=None,
                    op0=mybir.AluOpType.is_equal,
                )
                for k in range(4):
                    nc.tensor.matmul(
                        out=sumsT_ps[:, k * 512 : (k + 1) * 512],
                        lhsT=fn,
                        rhs=oh[:, k * 512 : (k + 1) * 512],
                        start=(t == 0),
                        stop=(t == T - 1),
                    )
                # transpose feat tile for phase 3
                trp = ps_tr.tile([P, P], f32r, tag="trp")
                nc.tensor.transpose(out=trp[:], in_=fn, identity=ident_t[:])
                nc.scalar.copy(out=featT[:, t * P : (t + 1) * P], in_=trp[:])

            # ---------- collective: AllReduce the [D, C] sums ----------
            sumsT_sb = pers.tile([P, C], f32)
            nc.scalar.copy(out=sumsT_sb[:], in_=sumsT_ps[:])
            nc.sync.dma_start(out=ar_in[:, :], in_=sumsT_sb[:])
            nc.gpsimd.collective_compute(
                "AllReduce",
                mybir.AluOpType.add,
                ins=[ar_in[:, :]],
                outs=[ar_out[:, :]],
                replica_groups=[list(range(NCORES))],
            )
            redT = pers.tile([P, C], f32r)
            nc.sync.dma_start(out=redT[:], in_=ar_out[:, :].bitcast(f32r))

            # ---------- phase 2: normalize prototypes, build table + protoT ----------
            for ct in range(CT):
                nat = nat_all[:, ct * P : (ct + 1) * P]
                trp = ps_tr.tile([P, P], f32r, tag="trp")
                nc.tensor.transpose(
                    out=trp[:],
                    in_=redT[:, ct * P : (ct + 1) * P],
                    identity=ident_t[:],
                )
                nc.scalar.copy(out=nat[:], in_=trp[:])
                sq = work.tile([P, P], f32, tag="sq")
                nc.vector.scalar_tensor_tensor(
                    out=sq[:],
                    in0=nat[:].bitcast(f32),
                    scalar=1.0,
                    in1=nat[:].bitcast(f32),
                    op0=mybir.AluOpType.mult,
                    op1=mybir.AluOpType.mult,
                    accum_out=norm2_all[:, ct : ct + 1],
                )
                nc.vector.tensor_scalar(
                    out=pres_all[:, ct : ct + 1],
                    in0=norm2_all[:, ct : ct + 1],
                    scalar1=float(EPS_PRESENT),
                    scalar2=None,
                    op0=mybir.AluOpType.is_gt,
                )

            # invnorm = exp(-0.5 * ln(max(norm2, eps)))  (rsqrt via ln/exp tables)
            n2c = small.tile([P, CT], f32, tag="n2c")
            nc.vector.tensor_scalar(
                out=n2c[:],
                in0=norm2_all[:],
                scalar1=float(EPS_NORM2),
                scalar2=None,
                op0=mybir.AluOpType.max,
            )
            lnv = small.tile([P, CT], f32, tag="lnv")
            nc.scalar.activation(
                out=lnv[:], in_=n2c[:], func=mybir.ActivationFunctionType.Ln
            )
            invn = small.tile([P, CT], f32, tag="invn")
            nc.scalar.activation(
                out=invn[:],
                in_=lnv[:],
                func=mybir.ActivationFunctionType.Exp,
                scale=-0.5,
            )

            for ct in range(CT):
                nat = nat_all[:, ct * P : (ct + 1) * P]
                nc.vector.tensor_scalar(
                    out=nat[:],
                    in0=nat[:],
                    scalar1=invn[:, ct : ct + 1],
                    scalar2=None,
                    op0=mybir.AluOpType.mult,
                )
                nc.sync.dma_start(out=table[ct * P : (ct + 1) * P, :].bitcast(f32r), in_=nat[:])
                trp = ps_tr.tile([P, P], f32r, tag="trp")
                nc.tensor.transpose(out=trp[:], in_=nat[:], identity=ident_t[:])
                nc.scalar.copy(out=protoT[:, ct * P : (ct + 1) * P], in_=trp[:])

            # n_absent = C - sum_c present  (partition reduce via ones matmuls)
            pres_sum = small.tile([P, 1], f32, tag="pres_sum")
            nc.vector.tensor_reduce(
                out=pres_sum[:],
                in_=pres_all[:],
                axis=mybir.AxisListType.X,
                op=mybir.AluOpType.add,
            )
            tp_ps = ps_acc.tile([1, 1], f32, tag="tp")
            nc.tensor.matmul(
                out=tp_ps[:1],
                lhsT=ones_col_t[:],
                rhs=pres_sum[:],
                start=True,
                stop=True,
            )
            tp_sb = small.tile([1, 1], f32, tag="tp_sb")
            nc.scalar.copy(out=tp_sb[:1], in_=tp_ps[:1])
            nab_ps = ps_acc.tile([P, 1], f32, tag="nab")
            nc.tensor.matmul(
                out=nab_ps[:],
                lhsT=ones_row_t[:1],
                rhs=tp_sb[:1],
                start=True,
                stop=True,
            )
            n_abs = pers.tile([P, 1], f32)
            nc.vector.tensor_scalar(
                out=n_abs[:],
                in0=nab_ps[:],
                scalar1=-1.0,
                scalar2=float(C),
                op0=mybir.AluOpType.mult,
                op1=mybir.AluOpType.add,
            )

            phase12_psum.close()
            ps_logp_cm = tc.tile_pool(name="psum_log", bufs=2, space="PSUM")
            ps_logp = ps_logp_cm.__enter__()

            # ---------- gather prototype rows by label (one row per partition
            # per indirect DMA; 64 of them, overlapped with phase-3 compute) ----
            for t in range(T):
                nc.gpsimd.indirect_dma_start(
                    out=g_all[:, t * P : (t + 1) * P],
                    out_offset=None,
                    in_=table[:, :],
                    in_offset=bass.IndirectOffsetOnAxis(
                        ap=lab_all[:, t : t + 1], axis=0
                    ),
                )

            # ---------- phase 3: logits, exp-sum, pos dot ----------
            for t in range(T):
                logp = ps_logp.tile([P, C], f32, tag="logp")
                for k in range(4):
                    nc.tensor.matmul(
                        out=logp[:, k * 512 : (k + 1) * 512],
                        lhsT=featT[:, t * P : (t + 1) * P],
                        rhs=protoT[:, k * 512 : (k + 1) * 512],
                        start=True,
                        stop=True,
                    )
                e_scr = work.tile([P, C], f32, tag="e_scr")
                nc.scalar.activation(
                    out=e_scr[:],
                    in_=logp[:],
                    func=mybir.ActivationFunctionType.Exp,
                    accum_out=denom_all[:, t : t + 1],
                )
                g = g_all[:, t * P : (t + 1) * P]
                prod = work.tile([P, P], f32, tag="prod")
                nc.vector.scalar_tensor_tensor(
                    out=prod[:],
                    in0=featn[:, t * P : (t + 1) * P].bitcast(f32),
                    scalar=1.0,
                    in1=g,
                    op0=mybir.AluOpType.mult,
                    op1=mybir.AluOpType.mult,
                    accum_out=l_all[:, t : t + 1],
                )
                prod2 = work.tile([P, P], f32, tag="prod2")
                nc.vector.scalar_tensor_tensor(
                    out=prod2[:],
                    in0=g,
                    scalar=1.0,
                    in1=g,
                    op0=mybir.AluOpType.mult,
                    op1=mybir.AluOpType.mult,
                    accum_out=zsq_all[:, t : t + 1],
                )

            # ---------- tail: p = z*exp(l)/(denom - n_abs) + (1-z) ----------
            pos = small.tile([P, T], f32, tag="pos")
            nc.scalar.activation(
                out=pos[:], in_=l_all[:], func=mybir.ActivationFunctionType.Exp
            )
            z = small.tile([P, T], f32, tag="z")
            nc.vector.tensor_scalar(
                out=z[:],
                in0=zsq_all[:],
                scalar1=0.5,
                scalar2=None,
                op0=mybir.AluOpType.is_gt,
            )
            dc = small.tile([P, T], f32, tag="dc")
            nc.vector.tensor_scalar(
                out=dc[:],
                in0=denom_all[:],
                scalar1=n_abs[:, :1],
                scalar2=None,
                op0=mybir.AluOpType.subtract,
            )
            rd = small.tile([P, T], f32, tag="rd")
            nc.vector.reciprocal(out=rd[:], in_=dc[:])
            t1 = small.tile([P, T], f32, tag="t1")
            nc.vector.tensor_tensor(
                out=t1[:], in0=pos[:], in1=rd[:], op=mybir.AluOpType.mult
            )
            t2 = small.tile([P, T], f32, tag="t2")
            nc.vector.tensor_tensor(
                out=t2[:], in0=t1[:], in1=z[:], op=mybir.AluOpType.mult
            )
            omz = small.tile([P, T], f32, tag="omz")
            nc.vector.tensor_scalar(
                out=omz[:],
                in0=z[:],
                scalar1=-1.0,
                scalar2=1.0,
                op0=mybir.AluOpType.mult,
                op1=mybir.AluOpType.add,
            )
            p_all = small.tile([P, T], f32, tag="p_all")
            nc.vector.tensor_tensor(
                out=p_all[:], in0=t2[:], in1=omz[:], op=mybir.AluOpType.add
            )
            out_r = out_p.rearrange("(t u) x -> u t x", u=P)
            nc.sync.dma_start(
                out=out_r[:, :, :], in_=p_all[:].unsqueeze(-1)
            )
            ps_logp_cm.__exit__(None, None, None)

    nc.compile()
    return nc


_NC_CACHE = None


def _get_nc():
    global _NC_CACHE
    if _NC_CACHE is None:
        _NC_CACHE = build_kernel()
    return _NC_CACHE


def prepare_inputs(feat_q, main_labels, sub_labels):
    feat_q = np.ascontiguousarray(feat_q, dtype=np.float32)
    main_labels = np.ascontiguousarray(main_labels, dtype=np.int32)
    sub_labels = np.ascontiguousarray(sub_labels, dtype=np.int32)

    seg = (main_labels + 4096 * sub_labels).astype(np.float32)  # never matches iota
    iota = np.tile(np.arange(C, dtype=np.int16)[None, :], (P, 1))
    ident = np.eye(P, dtype=np.float32)
    ones_col = np.ones((P, 1), dtype=np.float32)
    ones_row = np.ones((1, P), dtype=np.float32)

    in_maps = []
    for c in range(NCORES):
        sl = slice(c * NS, (c + 1) * NS)
        in_maps.append(
            {
                "feat": feat_q[sl],
                "seg": seg[sl, None],
                "labels": main_labels[sl, None],
                "iota": iota,
                "ident": ident,
                "ones_col": ones_col,
                "ones_row": ones_row,
            }
        )
    return in_maps


def run(feat_q, main_labels, sub_labels, trace=False, **kw):
    nc = _get_nc()
    in_maps = prepare_inputs(feat_q, main_labels, sub_labels)
    res = run_bass_kernel_spmd(
        nc, in_maps, core_ids=list(range(NCORES)), trace=trace, **kw
    )
    p = np.concatenate([res.results[c]["out_p"][:, 0] for c in range(NCORES)])
    return p, res


def kernel(feat_q, main_labels, sub_labels):
    p, _ = run(feat_q, main_labels, sub_labels)
    return p.astype(np.float32)
